# revision 29
# baseline (speedup 1.0000x reference)
"""Scatter-add of active-site feature rows into a dense (B, L, C) output,
distributed over 8 NeuronCores (data-parallel over the batch axis).

Core m owns flat output positions [m*8192, (m+1)*8192). Positions are
mapped to (group g, partition p, lane j) via  local = g*128*G + p*G + j
(p<128, j<G), so a group's output tile [128 partitions, G*512] stores to
DRAM with one contiguous G*2KB run per partition. On the host, rows are
bucketed by (core, g, j) "block" and padded to a uniform capacity Kc (the
runtime max block occupancy, rounded up to a multiple of 32 for DMA engine
fan-out); the lane count G is chosen per input to minimize Kc. On device
each block's [128, 512] output tile is a one-hot matmul

    out[p, c] = sum_k 1{lidx[k] == p} * feats[k, c]

which sums duplicate indices in fp32 PSUM and writes exact zeros for
untouched positions — every output element is produced by the kernel.

Features travel as bf16 (halves HBM read traffic; fp32 PSUM accumulation
keeps duplicate sums exact, total rel-err ~1e-3). Blocks of a group land
in one multi-bank PSUM tile that is copied to SBUF in a single wide copy,
alternating between the vector and scalar engines to keep either off the
critical path.
"""

import os

import numpy as np

import concourse.bacc as bacc
import concourse.mybir as mybir
import concourse.tile as tile
from concourse.bass_utils import run_bass_kernel_spmd

N_CORES = 8
B = 16
L = 4096
C = 512
POS_PER_CORE = B * L // N_CORES  # 8192
NBLK = 64  # blocks per core

G_ENV = os.environ.get("K_G")  # force a specific G (testing only)
FBUFS = int(os.environ.get("K_FBUFS", "0"))
OBUFS = int(os.environ.get("K_OBUFS", "0"))
MM_DTYPE = os.environ.get("K_MM_DTYPE", "bf16")  # bf16 | fp16 | float32
COPY_ENG = os.environ.get("K_COPY", "mix")  # dve | act | mix
STORE_MIX = int(os.environ.get("K_STORE_MIX", "0"))  # every Nth store on sync ring
KC_ENV = os.environ.get("K_KC")  # force Kc (testing only)
# Measured on this problem: uniform superblock loads, descriptor splitting
# (max_dma_last_dim), and ramp-shaped load segments (K_RAMP=1) were all
# neutral-to-worse vs plain per-group loads; splitting with Kc=88 wedged
# the device. Defaults keep the best-measured configuration.
RAMP = int(os.environ.get("K_RAMP", "0"))
# Buffer depths per G, sized to fit SBUF (ft is G*C*2B, ot G*C*4B per partition)
_BUFS = {2: (12, 10), 4: (10, 8), 8: (6, 4)}

_PROGRAM_CACHE: dict = {}


def _build_program(CH: int, Kc: int, G: int, FBUFS: int, OBUFS: int):
    NGRP = 64 // G
    CB = min(G, 4)  # blocks per PSUM tile (4 banks max -> double buffering)
    NCHUNK = G // CB  # psum chunks per group
    f32 = mybir.dt.float32
    fdt = {
        "bf16": mybir.dt.bfloat16,
        "fp16": mybir.dt.float16,
    }.get(MM_DTYPE, f32)
    nc = bacc.Bacc(
        "TRN2",
        target_bir_lowering=False,
        debug=False,
        enable_asserts=False,
        num_devices=N_CORES,
    )
    feats_d = [
        nc.dram_tensor(f"feats{ch}", [Kc, NBLK * C], fdt, kind="ExternalInput")
        for ch in range(CH)
    ]
    lidx_d = nc.dram_tensor("lidx", [Kc, CH * NBLK], f32, kind="ExternalInput")
    iota_d = nc.dram_tensor("iota", [128, 128], f32, kind="ExternalInput")
    out_d = nc.dram_tensor("out", [POS_PER_CORE, C], f32, kind="ExternalOutput")

    eq = mybir.AluOpType.is_equal

    with tile.TileContext(nc) as tc:
        with (
            tc.tile_pool(name="const", bufs=1) as constp,
            tc.tile_pool(name="mpool", bufs=int(os.environ.get("K_MBUFS", "6"))) as mpool,
            tc.tile_pool(name="fpool", bufs=FBUFS) as fpool,
            tc.tile_pool(name="opool", bufs=OBUFS) as opool,
            tc.tile_pool(name="psum", bufs=8 // CB, space="PSUM") as pspool,
        ):
            iota_t = constp.tile([128, 128], f32)
            lidx_t = constp.tile([Kc, CH * NBLK], f32)
            if not RAMP:
                nc.sync.dma_start(iota_t[:], iota_d.ap())
                nc.sync.dma_start(lidx_t[:], lidx_d.ap())

            # out viewed as [g, p, j, c]: row = g*128*G + p*G + j
            out_v = out_d.ap().rearrange("(g p j) c -> g p (j c)", p=128, j=G)
            # Load segments: groups covered per DMA instruction. Ramp-shaped
            # when RAMP: [1, 1, 2, 4, 4, ...] — consts dispatch after the
            # second segment so the first ft transfer starts immediately.
            if RAMP and NGRP >= 8:
                plan = [1, 1, 2]
            else:
                plan = [1] * NGRP
            segs = []
            g0 = 0
            for s in plan:
                if g0 >= NGRP:
                    break
                segs.append((g0, s))
                g0 += s
            while g0 < NGRP:
                s = min(4, NGRP - g0)
                segs.append((g0, s))
                g0 += s
            seg_of = {}
            for si, (gs, sp) in enumerate(segs):
                for g in range(gs, gs + sp):
                    seg_of[g] = (si, gs, sp)

            sftiles: dict = {}
            copy_idx = 0
            for g in range(NGRP):
                si, gs, sp = seg_of[g]
                if g == gs:
                    sftiles = {}
                    for ch in range(CH):
                        sft = fpool.tile([Kc, sp * G * C], fdt, tag="ft")
                        nc.sync.dma_start(
                            sft[:],
                            feats_d[ch].ap()[:, gs * G * C : (gs + sp) * G * C],
                        )
                        sftiles[ch] = sft
                    if RAMP and si == 1:
                        nc.sync.dma_start(iota_t[:], iota_d.ap())
                        nc.sync.dma_start(lidx_t[:], lidx_d.ap())
                ftiles = [
                    sftiles[ch][:, (g - gs) * G * C : (g - gs + 1) * G * C]
                    for ch in range(CH)
                ]
                ot = opool.tile([128, G * C], f32)
                for cb in range(NCHUNK):
                    ps = pspool.tile([128, CB * C], f32)
                    for jj in range(CB):
                        j = cb * CB + jj
                        b = g * G + j
                        for ch in range(CH):
                            m = mpool.tile([Kc, 128], fdt, tag="m")
                            nc.vector.tensor_scalar(
                                m[:],
                                iota_t[:Kc, :],
                                lidx_t[:, ch * NBLK + b : ch * NBLK + b + 1],
                                None,
                                op0=eq,
                            )
                            nc.tensor.matmul(
                                ps[:, jj * C : (jj + 1) * C],
                                m[:],
                                ftiles[ch][:, j * C : (j + 1) * C],
                                start=(ch == 0),
                                stop=(ch == CH - 1),
                            )
                    if COPY_ENG == "dve" or (COPY_ENG == "mix" and copy_idx % 2 == 0):
                        nc.vector.tensor_copy(
                            ot[:, cb * CB * C : (cb + 1) * CB * C], ps[:]
                        )
                    else:
                        nc.scalar.copy(ot[:, cb * CB * C : (cb + 1) * CB * C], ps[:])
                    copy_idx += 1
                # store on the second HWDGE ring (ACT) to decouple from loads;
                # optionally rebalance a fraction onto the sync ring
                if STORE_MIX and g % STORE_MIX == STORE_MIX - 1:
                    nc.sync.dma_start(out_v[g], ot[:])
                else:
                    nc.scalar.dma_start(out_v[g], ot[:])

    nc.compile()
    return nc


def _block_decomposition(idx, G):
    core = idx >> 13  # // 8192
    local = idx & 8191
    g = local // (128 * G)  # position group
    rem = local % (128 * G)
    p = rem // G  # partition (position G-tuple)
    j = rem % G  # lane within tuple
    blk = g * G + j  # block id within core, 0..63
    gblk = core * NBLK + blk  # global block id, 0..511
    counts = np.bincount(gblk, minlength=N_CORES * NBLK)
    K = int(counts.max())
    CH = (K + 127) // 128
    Kc = -(-K // CH)  # ceil
    # Multiple of 32 keeps the HWDGE descriptor fan-out balanced across all
    # 16 SDMA engines (measured: Kc=92 concentrates loads on 4 engines and
    # costs +80 us; Kc=96 spreads them).
    Kc = (Kc + 31) & ~31
    if KC_ENV is not None and CH == 1 and int(KC_ENV) >= K:
        Kc = int(KC_ENV)
    return gblk, p, CH, Kc


def _prepare_inputs(input_features, site_indices):
    feats = np.ascontiguousarray(np.asarray(input_features, dtype=np.float32))
    idx = np.asarray(site_indices).astype(np.int64)
    n = idx.shape[0]
    assert feats.shape == (n, C)

    # The block composition (hence the padded capacity Kc) depends on the
    # lane count G; pick the G that minimizes transferred bytes for this
    # input, preferring larger DMA runs on ties.
    if G_ENV is not None:
        G = int(G_ENV)
        gblk, lpos, CH, Kc = _block_decomposition(idx, G)
    else:
        best = None
        for cand in (4, 2, 8):
            gblk_c, lpos_c, CH_c, Kc_c = _block_decomposition(idx, cand)
            if best is None or CH_c * Kc_c < best[0] * best[1]:
                best = (CH_c, Kc_c, cand, gblk_c, lpos_c)
        CH, Kc, G, gblk, lpos = best

    order = np.argsort(gblk, kind="stable")
    counts = np.bincount(gblk, minlength=N_CORES * NBLK)

    starts = np.zeros(N_CORES * NBLK, dtype=np.int64)
    np.cumsum(counts[:-1], out=starts[1:])
    slot = np.arange(n, dtype=np.int64) - np.repeat(starts, counts)

    g_sorted = gblk[order]
    core_s = g_sorted // NBLK
    blk_s = g_sorted % NBLK
    ch_s = slot // Kc
    k_s = slot - ch_s * Kc

    if MM_DTYPE in ("bf16", "fp16"):
        if MM_DTYPE == "bf16":
            import ml_dtypes

            hdt = ml_dtypes.bfloat16
        else:
            hdt = np.float16
    else:
        hdt = np.float32
    feats_pack = np.zeros((N_CORES, CH, Kc, NBLK, C), dtype=hdt)
    feats_pack[core_s, ch_s, k_s, blk_s, :] = feats[order].astype(hdt)

    lidx_pack = np.full((N_CORES, CH, Kc, NBLK), -1.0, dtype=np.float32)
    lidx_pack[core_s, ch_s, k_s, blk_s] = lpos[order].astype(np.float32)

    iota = np.tile(np.arange(128, dtype=np.float32), (128, 1))

    in_maps = []
    for c in range(N_CORES):
        m = {
            "iota": iota,
            "lidx": lidx_pack[c].transpose(1, 0, 2).reshape(Kc, CH * NBLK),
        }
        for ch in range(CH):
            m[f"feats{ch}"] = feats_pack[c, ch].reshape(Kc, -1)
        in_maps.append(m)
    return in_maps, CH, Kc, G


def run(input_features, site_indices, trace: bool = False):
    in_maps, CH, Kc, G = _prepare_inputs(input_features, site_indices)
    fbufs = FBUFS or _BUFS[G][0]
    obufs = OBUFS or _BUFS[G][1]
    key = (CH, Kc, G, fbufs, obufs, MM_DTYPE, COPY_ENG, STORE_MIX, RAMP)
    if key not in _PROGRAM_CACHE:
        _PROGRAM_CACHE[key] = _build_program(CH, Kc, G, fbufs, obufs)
    nc = _PROGRAM_CACHE[key]
    res = run_bass_kernel_spmd(nc, in_maps, list(range(N_CORES)), trace=trace)
    out = np.concatenate([res.results[c]["out"] for c in range(N_CORES)], axis=0)
    return out.reshape(B, L, C), res


def kernel(input_features, site_indices, batch_size, length):
    assert int(batch_size) == B and int(length) == L
    out, _ = run(input_features, site_indices, trace=False)
    return out
